# revision 2
# baseline (speedup 1.0000x reference)
"""CoedgeConvLayer Trainium2 kernel, v2.

y = relu(x @ W_self + x[next] @ W_next + x[prev] @ W_prev + x[mate] @ W_mate + b)

Sharding: coedge rows data-parallel across 8 NeuronCores; the full bf16
feature table is replicated per core so neighbor gathers are local.  Per
512-row group: one 12-column indirect gather fetches the 3 neighbor streams
(rows land on partitions), the self stream is streamed k-major from a
host-pre-transposed per-core shard (no gather, no transpose), the gathered
subtiles are transposed on PE (bf16 transpose: 128 cycles) into PSUM and
batch-copied to SBUF by DVE, then 16 weight-stationary accumulating matmuls
(4 streams x 2 k-chunks x 2 dout-chunks, N=512 rows streamed) run into two
PSUM banks.  ReLU + per-partition bias is fused into the ACT copy to SBUF.
The output is produced transposed ([dout, rows], bf16) and un-transposed on
the host.
"""

import os

import numpy as np
import ml_dtypes

import concourse.bass as bass
from concourse import bacc
import concourse.mybir as mybir
import concourse.tile as tile
from concourse import bass_utils
from concourse.masks import make_identity

# Problem constants (hardcoded per harness contract).
N = 200000
D = 256
NCORES = 8
ROWS_PER_CORE = N // NCORES          # 25000
P = 128
GROUP = 512                          # rows per group (PSUM bank = 512 fp32)
SUBT = GROUP // P                    # 4 gathered subtiles per group
NGROUPS = (ROWS_PER_CORE + GROUP - 1) // GROUP   # 49
PAD_ROWS = NGROUPS * GROUP           # 25088
NNBR = 3                             # next, prev, mate
NSTREAMS = 4                         # + self
IDXCOLS = NNBR * SUBT                # 12 gather columns per group
KC = 2                               # k chunks (256 = 2*128)
DC = 2                               # dout chunks

GBUFS = int(os.environ.get("KERNEL_GBUFS", "30"))
ABLATE = os.environ.get("KERNEL_ABLATE", "")
XTBUFS = int(os.environ.get("KERNEL_XTBUFS", "3"))
PTBUFS = int(os.environ.get("KERNEL_PTBUFS", "3"))
PACCBUFS = int(os.environ.get("KERNEL_PACCBUFS", "4"))

BF16 = mybir.dt.bfloat16
NP_BF16 = ml_dtypes.bfloat16


def _build_nc(repeat=1):
    nc = bacc.Bacc("TRN2", debug=False, enable_partition_id=False)
    f32 = mybir.dt.float32
    feats = nc.dram_tensor("features", [N, D], BF16, kind="ExternalInput")
    feats_t = nc.dram_tensor("feats_t", [D, PAD_ROWS], BF16,
                             kind="ExternalInput")
    w = nc.dram_tensor("w", [P, NSTREAMS * KC * DC * P], BF16,
                       kind="ExternalInput")
    bias = nc.dram_tensor("bias", [P, DC], f32, kind="ExternalInput")
    idx = nc.dram_tensor("idx", [P, NGROUPS * IDXCOLS], mybir.dt.int32,
                         kind="ExternalInput")
    out = nc.dram_tensor("out", [D, PAD_ROWS], BF16, kind="ExternalOutput")

    feats_ap = feats.ap()
    # [P, KC, PAD_ROWS]: partition p holds k = m*128 + p columns of x^T.
    feats_t_ap = feats_t.ap().rearrange("(m p) n -> p m n", p=P)
    # [P, DC, PAD_ROWS]: partition q holds dout = d*128 + q rows of y^T.
    out_ap = out.ap().rearrange("(d p) n -> p d n", p=P)

    with tile.TileContext(nc) as tc:
        with (
            tc.tile_pool(name="const", bufs=1) as cpool,
            tc.tile_pool(name="gather", bufs=GBUFS) as gpool,
            tc.tile_pool(name="selfp", bufs=3) as xspool,
            tc.tile_pool(name="xt", bufs=XTBUFS) as xtpool,
            tc.tile_pool(name="outp", bufs=3) as opool,
            tc.tile_pool(name="pt", bufs=PTBUFS, space="PSUM") as ptpool,
            tc.tile_pool(name="pacc", bufs=PACCBUFS, space="PSUM") as paccpool,
        ):
            # Resident constants.
            w_sb = cpool.tile([P, NSTREAMS * KC * DC * P], BF16)
            nc.sync.dma_start(out=w_sb[:], in_=w.ap())
            bias_sb = cpool.tile([P, DC], f32)
            nc.sync.dma_start(out=bias_sb[:], in_=bias.ap())
            idx_sb = cpool.tile([P, NGROUPS * IDXCOLS], mybir.dt.int32)
            nc.sync.dma_start(out=idx_sb[:], in_=idx.ap())
            ident = cpool.tile([P, P], BF16)
            make_identity(nc, ident[:])
            dummy = None
            if ABLATE == "compute":
                dummy = cpool.tile([P, D], BF16)
                nc.gpsimd.memset(dummy[:], 0.25)
            # Priming transpose: folds the gpsimd-preamble wait into PE's
            # vector clock (single wait slot in the lowered LDWEIGHTS).
            pt0 = ptpool.tile([P, KC, GROUP], BF16, tag='pt')
            nc.tensor.transpose(pt0[:, 0, 0:P], ident[:], ident[:])

            def one_pass():
                for g in range(NGROUPS):
                    _group_body(g)

            def _group_body(g):
                g0 = g * GROUP
                # Neighbor gathers: rows on partitions.  One [P,1]-offset
                # single-row gather per (stream, subtile) — the only gather
                # form the HW SWDGE unroller handles (multi-column offset
                # APs silently stream consecutive rows from idx[p,0]).
                # xgs[s*SUBT+t][p, :] = feats[idx[p, g*12 + s*4 + t], :]
                #   = x[nbr_s[base + g*512 + t*128 + p], :]
                xgs = []
                if ABLATE != "compute":
                    for col in range(IDXCOLS):
                        xg = gpool.tile([P, D], BF16, tag="xg")
                        nc.gpsimd.indirect_dma_start(
                            out=xg[:],
                            out_offset=None,
                            in_=feats_ap,
                            in_offset=bass.IndirectOffsetOnAxis(
                                ap=idx_sb[:, g * IDXCOLS + col:
                                          g * IDXCOLS + col + 1], axis=0),
                        )
                        xgs.append(xg)
                else:
                    # compute-only ablation: transpose a resident dummy tile.
                    xgs = [dummy for _ in range(IDXCOLS)]
                if ABLATE == "gather":
                    # gather-only ablation: consume tiles cheaply, no compute.
                    junk = opool.tile([P, DC, GROUP], BF16)
                    for col in range(IDXCOLS):
                        nc.vector.tensor_copy(out=junk[:, 0, col:col + 1],
                                              in_=xgs[col][:, 0:1])
                    return
                # Self stream, already k-major: xs[p, m, j] = x[g0+j, m*128+p].
                xs = xspool.tile([P, KC, GROUP], BF16)
                nc.sync.dma_start(out=xs[:],
                                  in_=feats_t_ap[:, :, g0:g0 + GROUP])
                # Transpose neighbor subtiles on PE; batch-copy per stream.
                xt = xtpool.tile([P, NNBR, KC, GROUP], BF16)
                for s in range(NNBR):
                    pt = ptpool.tile([P, KC, GROUP], BF16, tag='pt')
                    for m in range(KC):
                        for t in range(SUBT):
                            nc.tensor.transpose(
                                pt[:, m, t * P:(t + 1) * P],
                                xgs[s * SUBT + t][:, m * P:(m + 1) * P],
                                ident[:])
                    nc.vector.tensor_copy(out=xt[:, s], in_=pt[:])
                # 16 accumulating matmuls: out_T[d*128+q, f] =
                #   sum_{st,m} w_chunk[st,m,d][p,q] * xT_st[m][p, f]
                outsb = opool.tile([P, DC, GROUP], BF16)
                for d in range(DC):
                    pacc = paccpool.tile([P, GROUP], f32)
                    nmm = NSTREAMS * KC
                    i = 0
                    for st in range(NSTREAMS):
                        for m in range(KC):
                            c = (st * KC + m) * DC + d
                            rhs = xs[:, m, :] if st == 0 else xt[:, st - 1, m, :]
                            nc.tensor.matmul(
                                pacc[:], lhsT=w_sb[:, c * P:(c + 1) * P],
                                rhs=rhs, start=(i == 0), stop=(i == nmm - 1))
                            i += 1
                    # Fused ReLU + per-partition bias on the PSUM->SBUF move.
                    nc.scalar.activation(
                        outsb[:, d, :], pacc[:],
                        mybir.ActivationFunctionType.Relu,
                        bias=bias_sb[:, d:d + 1])
                nc.sync.dma_start(out=out_ap[:, :, g0:g0 + GROUP],
                                  in_=outsb[:])

            if repeat == 1:
                one_pass()
            else:
                # Hardware loop: constant program size for any repeat count,
                # so the R-delta timing method is free of NEFF-size effects.
                with tc.For_i(0, repeat, 1):
                    one_pass()
    nc.compile()
    return nc


def _prepare_in_maps(features, next_indices, prev_indices, mate_indices,
                     W_self, b_self, W_next, b_next, W_prev, b_prev,
                     W_mate, b_mate):
    features = np.asarray(features, dtype=np.float32)
    feats_bf = np.ascontiguousarray(features.astype(NP_BF16))

    # lhsT chunk (st, m, d) = W_st[m*128:(m+1)*128, d*128:(d+1)*128].
    ws = [np.asarray(a, np.float32)
          for a in (W_self, W_next, W_prev, W_mate)]
    w_arr = np.empty((P, NSTREAMS * KC * DC, P), dtype=NP_BF16)
    for st in range(NSTREAMS):
        for m in range(KC):
            for d in range(DC):
                c = (st * KC + m) * DC + d
                w_arr[:, c, :] = ws[st][m * P:(m + 1) * P,
                                        d * P:(d + 1) * P].astype(NP_BF16)
    w_pack = np.ascontiguousarray(w_arr.reshape(P, NSTREAMS * KC * DC * P))

    b_tot = (np.asarray(b_self, np.float32) + np.asarray(b_next, np.float32)
             + np.asarray(b_prev, np.float32) + np.asarray(b_mate, np.float32))
    bias2 = np.ascontiguousarray(b_tot.reshape(DC, P).T.astype(np.float32))

    nbr = [np.asarray(next_indices), np.asarray(prev_indices),
           np.asarray(mate_indices)]

    in_maps = []
    for c in range(NCORES):
        base = c * ROWS_PER_CORE
        feats_t = np.zeros((D, PAD_ROWS), dtype=NP_BF16)
        feats_t[:, :ROWS_PER_CORE] = feats_bf[base:base + ROWS_PER_CORE].T
        # idx[p, g, s*SUBT+t] = nbr_s[base + g*512 + t*128 + p] (pad rows -> 0)
        idx_arr = np.zeros((P, NGROUPS, IDXCOLS), dtype=np.int32)
        for s, I in enumerate(nbr):
            loc = np.zeros(PAD_ROWS, dtype=np.int64)
            loc[:ROWS_PER_CORE] = I[base:base + ROWS_PER_CORE]
            idx_arr[:, :, s * SUBT:(s + 1) * SUBT] = (
                loc.reshape(NGROUPS, SUBT, P).transpose(2, 0, 1))
        in_maps.append({
            "features": feats_bf,
            "feats_t": np.ascontiguousarray(feats_t),
            "w": w_pack,
            "bias": bias2,
            "idx": np.ascontiguousarray(
                idx_arr.reshape(P, NGROUPS * IDXCOLS)),
        })
    return in_maps


def _assemble(core_outs):
    """Per-core transposed outputs [D, PAD_ROWS] -> full [N, D] float32."""
    out = np.concatenate(
        [np.asarray(o, dtype=np.float32).T[:ROWS_PER_CORE]
         for o in core_outs], axis=0)
    return np.ascontiguousarray(out)


def kernel(**inputs) -> np.ndarray:
    in_maps = _prepare_in_maps(**inputs)
    nc = _build_nc()
    res = bass_utils.run_bass_kernel_spmd(
        nc, in_maps, core_ids=list(range(NCORES)))
    return _assemble([r["out"] for r in res.results])
